# revision 8
# baseline (speedup 1.0000x reference)
"""BitLinear-1.58b Trainium2 kernel.

Computation (see BitLinear reference):
  scale = clip(mean(|W|), eps)                 (scalar)
  qw    = clip(round(W/scale), -1, 1)          (ternary)
  gamma = clip(max|x| per token, eps)
  qx    = clip(round(x * 128/gamma), -128, 127)
  y     = (qx @ qw^T + bias) * scale*gamma/128

Distribution: tokens (B*S = 8192) are data-parallel sharded 1024 per core
across 8 cores; the weight (and its quantization work) is replicated.
The scalar `scale` is computed on host before sharding (it is a global
mean over the full weight).  The weight is passed to each core already
transposed ([DIN, DOUT]) -- a pure host-side layout transformation -- so
the device quantizes it directly into the matmul-ready K-major layout.

All quantized values (qx in [-128,127], qw in {-1,0,1}) are exactly
representable in bf16, and every partial dot product is an integer with
magnitude <= 2048*128 < 2^24, so a bf16 matmul with fp32 PSUM
accumulation reproduces the reference arithmetic exactly.  Rounding uses
the fp32 magic-number trick (v + 1.5*2^23 - 1.5*2^23) which matches
round-half-to-even exactly for |v| < 2^22.  The fp32 bias is folded into
the matmul as three extra bf16 K-rows (b0+b1+b2 reconstructs the fp32
bias exactly) against an all-ones stationary column.

Weight quantization is organized in n-panels (DOUT blocks of 512) so the
first accumulation chains only depend on 1/4 of the weight, and within a
panel in k-quads ([128, 4, 512] chunks) to keep the elementwise
instruction count low.
"""

import numpy as np
import ml_dtypes

import concourse.bass as bass
import concourse.mybir as mybir
import concourse.tile as tile
from concourse import bacc
from concourse.bass_utils import run_bass_kernel_spmd

P = 128
DIN = 2048
DOUT = 2048
N_CORES = 8
TOK = 1024  # tokens per core
KT = DIN // P  # 16 k-tiles
MT = TOK // P  # 8 m-tiles
NW = 512  # matmul moving free dim (one PSUM bank)
NB = DOUT // NW  # 4 n-blocks
KQ = 4  # k-tiles per W quant chunk (quad)
NQ = KT // KQ  # 4 quads per panel

F32 = mybir.dt.float32
BF16 = mybir.dt.bfloat16
ALU = mybir.AluOpType
AFT = mybir.ActivationFunctionType

MAGIC = 12582912.0  # 1.5 * 2^23: fp32 round-to-nearest-even magic constant
EPS = 1e-5
Q = 128.0

_CACHE: dict = {}

# test harness hooks (set by test.py; harmless defaults for grading)
TRACE = False
LAST_RESULTS = None


def _build():
    nc = bacc.Bacc("TRN2", target_bir_lowering=False, debug=False)

    x_d = nc.dram_tensor("x", [TOK, DIN], F32, kind="ExternalInput")
    wt_d = nc.dram_tensor("wt", [DIN, DOUT], F32, kind="ExternalInput")
    bias3_d = nc.dram_tensor("bias3", [3, DOUT], BF16, kind="ExternalInput")
    # consts[:, 0] = 1/scale, consts[:, 1] = scale/128  (replicated per partition)
    consts_d = nc.dram_tensor("consts", [P, 2], F32, kind="ExternalInput")
    y_d = nc.dram_tensor("y", [TOK, DOUT], F32, kind="ExternalOutput")

    with tile.TileContext(nc) as tc:
        with (
            tc.tile_pool(name="const", bufs=1) as cpool,
            tc.tile_pool(name="wq", bufs=1) as wq_pool,
            tc.tile_pool(name="qxt", bufs=1) as qxt_pool,
            tc.tile_pool(name="xstage", bufs=2) as xstage,
            tc.tile_pool(name="wstage", bufs=3) as wstage,
            tc.tile_pool(name="xtmp", bufs=2) as xtmp,
            tc.tile_pool(name="qn", bufs=2) as qn_pool,
            tc.tile_pool(name="outp", bufs=6) as outp,
            tc.tile_pool(name="small", bufs=4) as small,
            tc.tile_pool(name="psum", bufs=8, space="PSUM") as psum_pool,
        ):
            consts = cpool.tile([P, 2], F32, tag="consts")
            nc.sync.dma_start(consts[:], consts_d[:])
            bias3 = cpool.tile([3, DOUT], BF16, tag="bias3")
            nc.sync.dma_start(bias3[:], bias3_d[:])
            ones3 = cpool.tile([3, P], BF16, tag="ones3")
            nc.vector.memset(ones3[:], 1.0)
            inv_scale = consts[:, 0:1]
            s128 = consts[:, 1:2]

            # ---- weight quantization: per (n-panel, k-quad) chunk ----
            # qwt[n][q] is [P, KQ, NW] bf16; rhs for (n, k) is qwt[n][k//KQ][:, k%KQ, :]
            qwt = [[None] * NQ for _ in range(NB)]

            def quant_w_quad(n, q):
                wch = wstage.tile([P, KQ, NW], F32, tag="wstage")
                src = wt_d[q * KQ * P : (q + 1) * KQ * P, n * NW : (n + 1) * NW]
                nc.sync.dma_start(wch[:], src.rearrange("(i p) j -> p i j", p=P))
                # t = w*(1/scale) + MAGIC  (in-place, ScalarE)
                nc.scalar.activation(
                    wch[:], wch[:], AFT.Copy, bias=MAGIC, scale=inv_scale
                )
                # clip in magic domain (in-place):  min(t, M+1) then max(., M-1)
                nc.vector.tensor_scalar(
                    wch[:], wch[:], MAGIC + 1.0, MAGIC - 1.0, op0=ALU.min, op1=ALU.max
                )
                # qw = t - MAGIC -> bf16 (exact ternary)
                qw = wq_pool.tile([P, KQ, NW], BF16, tag=f"qw_{n}_{q}")
                nc.vector.tensor_scalar(qw[:], wch[:], MAGIC, None, op0=ALU.subtract)
                qwt[n][q] = qw

            # ---- activation quantization per 128-token tile ----
            qxt = [None] * MT
            mscale = [None] * MT

            def quant_x_tile(m):
                xch = xstage.tile([P, DIN], F32, tag="xstage")
                nc.sync.dma_start(xch[:], x_d[m * P : (m + 1) * P, :])
                g0 = small.tile([P, 1], F32, tag="g0")
                nc.vector.tensor_reduce(
                    g0[:],
                    xch[:],
                    axis=mybir.AxisListType.X,
                    op=ALU.max,
                    apply_absolute_value=True,
                )
                gamma = small.tile([P, 1], F32, tag="gamma")
                nc.vector.tensor_scalar_max(gamma[:], g0[:], EPS)
                # r = 128/gamma == 1/(gamma/128); gamma/128 is exact, so a
                # correctly-rounded reciprocal reproduces fl(128/gamma)
                g128 = small.tile([P, 1], F32, tag="g128")
                nc.vector.tensor_scalar_mul(g128[:], gamma[:], 1.0 / Q)
                r = small.tile([P, 1], F32, tag="r")
                nc.vector.reciprocal(r[:], g128[:])
                ms = cpool.tile([P, 1], F32, tag=f"ms_{m}")
                nc.vector.tensor_scalar_mul(ms[:], gamma[:], s128)
                mscale[m] = ms
                # t = x*r + MAGIC ; clip in magic domain ; q = t - MAGIC -> bf16
                t = xtmp.tile([P, DIN], F32, tag="xtmp")
                nc.vector.tensor_scalar(
                    t[:], xch[:], r[:], MAGIC, op0=ALU.mult, op1=ALU.add
                )
                nc.vector.tensor_scalar(
                    t[:],
                    t[:],
                    MAGIC + (Q - 1.0),
                    MAGIC - Q,
                    op0=ALU.min,
                    op1=ALU.max,
                )
                qxn = qn_pool.tile([P, DIN], BF16, tag="qxn")
                nc.vector.tensor_scalar(qxn[:], t[:], MAGIC, None, op0=ALU.subtract)
                # transpose to [DIN, tok] layout: qt[p, k, j] = qxn[j, k*128+p]
                qt = qxt_pool.tile([P, KT, P], BF16, tag=f"qxt_{m}")
                nc.sync.dma_start_transpose(qt[:], qxn[:])
                qxt[m] = qt

            # interleave x-tile and W-panel quantization so both pipelines
            # start immediately and panel n is ready before its chains
            quant_x_tile(0)
            for q in range(NQ):
                quant_w_quad(0, q)
            quant_x_tile(1)
            quant_x_tile(2)
            for q in range(NQ):
                quant_w_quad(1, q)
            quant_x_tile(3)
            quant_x_tile(4)
            for q in range(NQ):
                quant_w_quad(2, q)
            quant_x_tile(5)
            quant_x_tile(6)
            quant_x_tile(7)
            for q in range(NQ):
                quant_w_quad(3, q)

            # ---- matmul chains + epilogue ----
            for n in range(NB):
                for m in range(MT):
                    ps = psum_pool.tile([P, NW], F32, tag="ps")
                    for k in range(KT):
                        nc.tensor.matmul(
                            ps[:],
                            qxt[m][:, k, :],
                            qwt[n][k // KQ][:, k % KQ, :],
                            start=(k == 0),
                            stop=False,
                        )
                    # bias: + ones3^T @ bias3 == bias broadcast to all tokens
                    nc.tensor.matmul(
                        ps[:],
                        ones3[:],
                        bias3[:, n * NW : (n + 1) * NW],
                        start=False,
                        stop=True,
                    )
                    o = outp.tile([P, NW], F32, tag="o")
                    nc.scalar.activation(
                        o[:], ps[:], AFT.Copy, bias=0.0, scale=mscale[m][:]
                    )
                    nc.sync.dma_start(
                        y_d[m * P : (m + 1) * P, n * NW : (n + 1) * NW], o[:]
                    )
    nc.compile()
    return nc


def kernel(x: np.ndarray, weight: np.ndarray, bias: np.ndarray) -> np.ndarray:
    global LAST_RESULTS
    B, S, _ = x.shape

    x2 = np.ascontiguousarray(x.reshape(B * S, DIN), dtype=np.float32)
    wt = np.ascontiguousarray(weight.T.astype(np.float32, copy=False))
    # bias as three bf16 rows reconstructing the fp32 bias exactly
    bias_f = bias.astype(np.float32, copy=False)
    b0 = bias_f.astype(ml_dtypes.bfloat16)
    r1 = (bias_f - b0.astype(np.float32)).astype(np.float32)
    b1 = r1.astype(ml_dtypes.bfloat16)
    r2 = (r1 - b1.astype(np.float32)).astype(np.float32)
    b2 = r2.astype(ml_dtypes.bfloat16)
    bias3 = np.ascontiguousarray(np.stack([b0, b1, b2], axis=0))
    # global scalar: computed on host before sharding (see sharding note)
    scale = np.float32(max(np.mean(np.abs(weight), dtype=np.float64), EPS))
    inv_scale = np.float32(1.0 / np.float64(scale))
    s128 = np.float32(scale) / np.float32(Q)  # exact (power-of-two divide)
    consts = np.empty((P, 2), dtype=np.float32)
    consts[:, 0] = inv_scale
    consts[:, 1] = s128

    if "nc" not in _CACHE:
        _CACHE["nc"] = _build()
    nc = _CACHE["nc"]

    in_maps = [
        {
            "x": x2[i * TOK : (i + 1) * TOK],
            "wt": wt,
            "bias3": bias3,
            "consts": consts,
        }
        for i in range(N_CORES)
    ]
    res = run_bass_kernel_spmd(
        nc,
        in_maps,
        list(range(N_CORES)),
        trace=TRACE,
        trace_cores=list(range(N_CORES)) if TRACE else None,
    )
    LAST_RESULTS = res
    out = np.concatenate([res.results[i]["y"] for i in range(N_CORES)], axis=0)
    return np.ascontiguousarray(out.reshape(B, S, DOUT).astype(np.float32, copy=False))


# revision 9
# speedup vs baseline: 1.1270x; 1.1270x over previous
"""BitLinear-1.58b Trainium2 kernel.

Computation (see BitLinear reference):
  scale = clip(mean(|W|), eps)                 (scalar)
  qw    = clip(round(W/scale), -1, 1)          (ternary)
  gamma = clip(max|x| per token, eps)
  qx    = clip(round(x * 128/gamma), -128, 127)
  y     = (qx @ qw^T + bias) * scale*gamma/128

Distribution: tokens (B*S = 8192) are data-parallel sharded 1024 per core
across 8 cores; the weight (and its quantization work) is replicated.
The scalar `scale` is computed on host before sharding (it is a global
mean over the full weight).  The weight is passed to each core already
transposed ([DIN, DOUT]) -- a pure host-side layout transformation -- so
the device quantizes it directly into the matmul-ready K-major layout.

All quantized values (qx in [-128,127], qw in {-1,0,1}) are exactly
representable in bf16, and every partial dot product is an integer with
magnitude <= 2048*128 < 2^24, so a bf16 matmul with fp32 PSUM
accumulation reproduces the reference arithmetic exactly.  Rounding uses
the fp32 magic-number trick (v + 1.5*2^23 - 1.5*2^23) which matches
round-half-to-even exactly for |v| < 2^22.  The fp32 bias is added by
pre-initializing each PSUM bank with the bias row (ScalarE copy) and
accumulating the matmul chain on top.

Weight quantization is organized in n-panels (DOUT blocks of 512) so the
first accumulation chains only depend on 1/4 of the weight, and within a
panel in k-quads ([128, 4, 512] chunks) to keep the elementwise
instruction count low.
"""

import numpy as np

import concourse.bass as bass
import concourse.mybir as mybir
import concourse.tile as tile
from concourse import bacc
from concourse.bass_utils import run_bass_kernel_spmd

P = 128
DIN = 2048
DOUT = 2048
N_CORES = 8
TOK = 1024  # tokens per core
KT = DIN // P  # 16 k-tiles
MT = TOK // P  # 8 m-tiles
NW = 512  # matmul moving free dim (one PSUM bank)
NB = DOUT // NW  # 4 n-blocks
KQ = 4  # k-tiles per W quant chunk (quad)
NQ = KT // KQ  # 4 quads per panel

F32 = mybir.dt.float32
BF16 = mybir.dt.bfloat16
ALU = mybir.AluOpType
AFT = mybir.ActivationFunctionType

MAGIC = 12582912.0  # 1.5 * 2^23: fp32 round-to-nearest-even magic constant
EPS = 1e-5
Q = 128.0

_CACHE: dict = {}

# test harness hooks (set by test.py; harmless defaults for grading)
TRACE = False
LAST_RESULTS = None


def _build():
    nc = bacc.Bacc("TRN2", target_bir_lowering=False, debug=False)

    x_d = nc.dram_tensor("x", [TOK, DIN], F32, kind="ExternalInput")
    wt_d = nc.dram_tensor("wt", [DIN, DOUT], F32, kind="ExternalInput")
    bias_d = nc.dram_tensor("biasrep", [P, DOUT], F32, kind="ExternalInput")
    # consts[:, 0] = 1/scale, consts[:, 1] = scale/128  (replicated per partition)
    consts_d = nc.dram_tensor("consts", [P, 2], F32, kind="ExternalInput")
    y_d = nc.dram_tensor("y", [TOK, DOUT], F32, kind="ExternalOutput")

    with tile.TileContext(nc) as tc:
        with (
            tc.tile_pool(name="const", bufs=1) as cpool,
            tc.tile_pool(name="wq", bufs=1) as wq_pool,
            tc.tile_pool(name="qxt", bufs=1) as qxt_pool,
            tc.tile_pool(name="xstage", bufs=2) as xstage,
            tc.tile_pool(name="wstage", bufs=3) as wstage,
            tc.tile_pool(name="xtmp", bufs=2) as xtmp,
            tc.tile_pool(name="qn", bufs=2) as qn_pool,
            tc.tile_pool(name="outp", bufs=6) as outp,
            tc.tile_pool(name="small", bufs=4) as small,
            tc.tile_pool(name="psum", bufs=8, space="PSUM") as psum_pool,
        ):
            consts = cpool.tile([P, 2], F32, tag="consts")
            nc.sync.dma_start(consts[:], consts_d[:])
            biasrep = cpool.tile([P, DOUT], F32, tag="biasrep")
            nc.sync.dma_start(biasrep[:], bias_d[:])
            inv_scale = consts[:, 0:1]
            s128 = consts[:, 1:2]

            # ---- weight quantization: per (n-panel, k-quad) chunk ----
            # qwt[n][q] is [P, KQ, NW] bf16; rhs for (n, k) is qwt[n][k//KQ][:, k%KQ, :]
            qwt = [[None] * NQ for _ in range(NB)]

            def quant_w_quad(n, q):
                wch = wstage.tile([P, KQ, NW], F32, tag="wstage")
                src = wt_d[q * KQ * P : (q + 1) * KQ * P, n * NW : (n + 1) * NW]
                nc.sync.dma_start(wch[:], src.rearrange("(i p) j -> p i j", p=P))
                # t = w*(1/scale) + MAGIC  (in-place, ScalarE)
                nc.scalar.activation(
                    wch[:], wch[:], AFT.Copy, bias=MAGIC, scale=inv_scale
                )
                # clip in magic domain (in-place):  min(t, M+1) then max(., M-1)
                nc.vector.tensor_scalar(
                    wch[:], wch[:], MAGIC + 1.0, MAGIC - 1.0, op0=ALU.min, op1=ALU.max
                )
                # qw = t - MAGIC -> bf16 (exact ternary)
                qw = wq_pool.tile([P, KQ, NW], BF16, tag=f"qw_{n}_{q}")
                nc.vector.tensor_scalar(qw[:], wch[:], MAGIC, None, op0=ALU.subtract)
                qwt[n][q] = qw

            # ---- activation quantization per 128-token tile ----
            qxt = [None] * MT
            mscale = [None] * MT

            def quant_x_tile(m):
                xch = xstage.tile([P, DIN], F32, tag="xstage")
                nc.sync.dma_start(xch[:], x_d[m * P : (m + 1) * P, :])
                g0 = small.tile([P, 1], F32, tag="g0")
                nc.vector.tensor_reduce(
                    g0[:],
                    xch[:],
                    axis=mybir.AxisListType.X,
                    op=ALU.max,
                    apply_absolute_value=True,
                )
                gamma = small.tile([P, 1], F32, tag="gamma")
                nc.vector.tensor_scalar_max(gamma[:], g0[:], EPS)
                # r = 128/gamma == 1/(gamma/128); gamma/128 is exact, so a
                # correctly-rounded reciprocal reproduces fl(128/gamma)
                g128 = small.tile([P, 1], F32, tag="g128")
                nc.vector.tensor_scalar_mul(g128[:], gamma[:], 1.0 / Q)
                r = small.tile([P, 1], F32, tag="r")
                nc.vector.reciprocal(r[:], g128[:])
                ms = cpool.tile([P, 1], F32, tag=f"ms_{m}")
                nc.vector.tensor_scalar_mul(ms[:], gamma[:], s128)
                mscale[m] = ms
                # t = x*r + MAGIC (ScalarE); clip in magic domain; q = t-MAGIC -> bf16
                t = xtmp.tile([P, DIN], F32, tag="xtmp")
                nc.scalar.activation(t[:], xch[:], AFT.Copy, bias=MAGIC, scale=r[:])
                nc.vector.tensor_scalar(
                    t[:],
                    t[:],
                    MAGIC + (Q - 1.0),
                    MAGIC - Q,
                    op0=ALU.min,
                    op1=ALU.max,
                )
                qxn = qn_pool.tile([P, DIN], BF16, tag="qxn")
                nc.vector.tensor_scalar(qxn[:], t[:], MAGIC, None, op0=ALU.subtract)
                # transpose to [DIN, tok] layout: qt[p, k, j] = qxn[j, k*128+p]
                qt = qxt_pool.tile([P, KT, P], BF16, tag=f"qxt_{m}")
                nc.sync.dma_start_transpose(qt[:], qxn[:])
                qxt[m] = qt

            # interleave x-tile and W-panel quantization so both pipelines
            # start immediately and panel n is ready before its chains
            quant_x_tile(0)
            quant_x_tile(1)
            for q in range(NQ):
                quant_w_quad(0, q)
            quant_x_tile(2)
            quant_x_tile(3)
            for q in range(NQ):
                quant_w_quad(1, q)
            quant_x_tile(4)
            quant_x_tile(5)
            for q in range(NQ):
                quant_w_quad(2, q)
            quant_x_tile(6)
            quant_x_tile(7)
            for q in range(NQ):
                quant_w_quad(3, q)

            # ---- matmul chains + epilogue ----
            for n in range(NB):
                for m in range(MT):
                    ps = psum_pool.tile([P, NW], F32, tag="ps")
                    # pre-load bias into PSUM; the chain accumulates on top
                    nc.scalar.activation(
                        ps[:],
                        biasrep[:, n * NW : (n + 1) * NW],
                        AFT.Copy,
                        bias=0.0,
                        scale=1.0,
                    )
                    for k in range(KT):
                        nc.tensor.matmul(
                            ps[:],
                            qxt[m][:, k, :],
                            qwt[n][k // KQ][:, k % KQ, :],
                            start=False,
                            stop=(k == KT - 1),
                            skip_group_check=True,
                        )
                    o = outp.tile([P, NW], F32, tag="o")
                    nc.scalar.activation(
                        o[:], ps[:], AFT.Copy, bias=0.0, scale=mscale[m][:]
                    )
                    nc.sync.dma_start(
                        y_d[m * P : (m + 1) * P, n * NW : (n + 1) * NW], o[:]
                    )
    nc.compile()
    return nc


def kernel(x: np.ndarray, weight: np.ndarray, bias: np.ndarray) -> np.ndarray:
    global LAST_RESULTS
    B, S, _ = x.shape

    x2 = np.ascontiguousarray(x.reshape(B * S, DIN), dtype=np.float32)
    wt = np.ascontiguousarray(weight.T.astype(np.float32, copy=False))
    biasrep = np.ascontiguousarray(
        np.broadcast_to(bias.astype(np.float32, copy=False), (P, DOUT))
    )
    # global scalar: computed on host before sharding (see sharding note)
    scale = np.float32(max(np.mean(np.abs(weight), dtype=np.float64), EPS))
    inv_scale = np.float32(1.0 / np.float64(scale))
    s128 = np.float32(scale) / np.float32(Q)  # exact (power-of-two divide)
    consts = np.empty((P, 2), dtype=np.float32)
    consts[:, 0] = inv_scale
    consts[:, 1] = s128

    if "nc" not in _CACHE:
        _CACHE["nc"] = _build()
    nc = _CACHE["nc"]

    in_maps = [
        {
            "x": x2[i * TOK : (i + 1) * TOK],
            "wt": wt,
            "biasrep": biasrep,
            "consts": consts,
        }
        for i in range(N_CORES)
    ]
    res = run_bass_kernel_spmd(
        nc,
        in_maps,
        list(range(N_CORES)),
        trace=TRACE,
        trace_cores=list(range(N_CORES)) if TRACE else None,
    )
    LAST_RESULTS = res
    out = np.concatenate([res.results[i]["y"] for i in range(N_CORES)], axis=0)
    return np.ascontiguousarray(out.reshape(B, S, DOUT).astype(np.float32, copy=False))


# revision 19
# speedup vs baseline: 1.1870x; 1.0532x over previous
"""BitLinear-1.58b Trainium2 kernel.

Computation (see BitLinear reference):
  scale = clip(mean(|W|), eps)                 (scalar)
  qw    = clip(round(W/scale), -1, 1)          (ternary)
  gamma = clip(max|x| per token, eps)
  qx    = clip(round(x * 128/gamma), -128, 127)
  y     = (qx @ qw^T + bias) * scale*gamma/128

Distribution: tokens (B*S = 8192) are data-parallel sharded 1024 per core
across 8 cores; the weight (and its quantization work) is replicated.
The scalar `scale` is computed on host before sharding (it is a global
mean over the full weight).  The weight is passed to each core already
transposed ([DIN, DOUT]) -- a pure host-side layout transformation -- so
the device quantizes it directly into the matmul-ready K-major layout.

All quantized values (qx in [-128,127], qw in {-1,0,1}) are exactly
representable in bf16, and every partial dot product is an integer with
magnitude <= 2048*128 < 2^24, so a bf16 matmul with fp32 PSUM
accumulation reproduces the reference arithmetic exactly.  Rounding uses
the fp32 magic-number trick (v + 1.5*2^23 - 1.5*2^23) which matches
round-half-to-even exactly for |v| < 2^22.  The fp32 bias is added by
pre-initializing each PSUM bank with the bias row (ScalarE copy) and
accumulating the matmul chain on top.

Weight quantization is organized in n-panels (DOUT blocks of 512) so the
first accumulation chains only depend on 1/4 of the weight, and within a
panel in k-quads ([128, 4, 512] chunks) to keep the elementwise
instruction count low.
"""

import numpy as np

import concourse.bass as bass
import concourse.mybir as mybir
import concourse.tile as tile
from concourse import bacc
from concourse.bass_utils import run_bass_kernel_spmd

P = 128
DIN = 2048
DOUT = 2048
N_CORES = 8
TOK = 1024  # tokens per core
KT = DIN // P  # 16 k-tiles
MT = TOK // P  # 8 m-tiles
NW = 512  # matmul moving free dim (one PSUM bank)
NB = DOUT // NW  # 4 n-blocks
KQ = 4  # k-tiles per W quant chunk (quad)
NQ = KT // KQ  # 4 quads per panel

F32 = mybir.dt.float32
BF16 = mybir.dt.bfloat16
ALU = mybir.AluOpType
AFT = mybir.ActivationFunctionType

MAGIC = 12582912.0  # 1.5 * 2^23: fp32 round-to-nearest-even magic constant
EPS = 1e-5
Q = 128.0

_CACHE: dict = {}

# test harness hooks (set by test.py; harmless defaults for grading)
TRACE = False
LAST_RESULTS = None


def _build():
    nc = bacc.Bacc("TRN2", target_bir_lowering=False, debug=False)

    x_d = nc.dram_tensor("x", [TOK, DIN], F32, kind="ExternalInput")
    wt_d = nc.dram_tensor("wt", [DIN, DOUT], F32, kind="ExternalInput")
    bias_d = nc.dram_tensor("biasrep", [P, DOUT], F32, kind="ExternalInput")
    # consts[:, 0] = 1/scale, consts[:, 1] = scale/128  (replicated per partition)
    consts_d = nc.dram_tensor("consts", [P, 2], F32, kind="ExternalInput")
    y_d = nc.dram_tensor("y", [TOK, DOUT], F32, kind="ExternalOutput")

    with tile.TileContext(nc) as tc:
        with (
            tc.tile_pool(name="const", bufs=1) as cpool,
            tc.tile_pool(name="wq", bufs=1) as wq_pool,
            tc.tile_pool(name="qxt", bufs=1) as qxt_pool,
            tc.tile_pool(name="xstage", bufs=2) as xstage,
            tc.tile_pool(name="wstage", bufs=3) as wstage,
            tc.tile_pool(name="xtmp", bufs=2) as xtmp,
            tc.tile_pool(name="qn", bufs=2) as qn_pool,
            tc.tile_pool(name="outp", bufs=6) as outp,
            tc.tile_pool(name="small", bufs=4) as small,
            tc.tile_pool(name="psum", bufs=4, space="PSUM") as psum_pool,
        ):
            # x tile 0 load is emitted first: it heads the serial critical
            # path (load -> reduce -> quant -> transpose -> first matmul)
            x0 = xstage.tile([P, DIN], F32, tag="xstage")
            nc.sync.dma_start(x0[:], x_d[0:P, :])
            consts = cpool.tile([P, 2], F32, tag="consts")
            nc.sync.dma_start(consts[:], consts_d[:])
            biasrep = cpool.tile([P, DOUT], F32, tag="biasrep")
            nc.sync.dma_start(biasrep[:], bias_d[:])
            inv_scale = consts[:, 0:1]
            s128 = consts[:, 1:2]

            # ---- weight quantization: per (n-panel, k-quad) chunk ----
            # qwt[n][q] is [P, KQ, NW] bf16; rhs for (n, k) is qwt[n][k//KQ][:, k%KQ, :]
            qwt = [[None] * NQ for _ in range(NB)]

            def quant_w_quad(n, q):
                wch = wstage.tile([P, KQ, NW], F32, tag="wstage")
                src = wt_d[q * KQ * P : (q + 1) * KQ * P, n * NW : (n + 1) * NW]
                nc.sync.dma_start(wch[:], src.rearrange("(i p) j -> p i j", p=P))
                # t = w*(1/scale) + MAGIC  (in-place, ScalarE)
                nc.scalar.activation(
                    wch[:], wch[:], AFT.Copy, bias=MAGIC, scale=inv_scale
                )
                # clip in magic domain (in-place):  min(t, M+1) then max(., M-1)
                nc.vector.tensor_scalar(
                    wch[:], wch[:], MAGIC + 1.0, MAGIC - 1.0, op0=ALU.min, op1=ALU.max
                )
                # qw = t - MAGIC -> bf16 (exact ternary)
                qw = wq_pool.tile([P, KQ, NW], BF16, tag=f"qw_{n}_{q}")
                nc.vector.tensor_scalar(qw[:], wch[:], MAGIC, None, op0=ALU.subtract)
                qwt[n][q] = qw

            # ---- activation quantization per 128-token tile ----
            qxt = [None] * MT
            mscale = [None] * MT

            def quant_x_tile(m, xch=None):
                if xch is None:
                    xch = xstage.tile([P, DIN], F32, tag="xstage")
                    nc.sync.dma_start(xch[:], x_d[m * P : (m + 1) * P, :])
                g0 = small.tile([P, 1], F32, tag="g0")
                nc.vector.tensor_reduce(
                    g0[:],
                    xch[:],
                    axis=mybir.AxisListType.X,
                    op=ALU.max,
                    apply_absolute_value=True,
                )
                gamma = small.tile([P, 1], F32, tag="gamma")
                nc.vector.tensor_scalar_max(gamma[:], g0[:], EPS)
                # r = 128/gamma == 1/(gamma/128); gamma/128 is exact, so a
                # correctly-rounded reciprocal reproduces fl(128/gamma)
                g128 = small.tile([P, 1], F32, tag="g128")
                nc.vector.tensor_scalar_mul(g128[:], gamma[:], 1.0 / Q)
                r = small.tile([P, 1], F32, tag="r")
                nc.vector.reciprocal(r[:], g128[:])
                ms = cpool.tile([P, 1], F32, tag=f"ms_{m}")
                nc.vector.tensor_scalar_mul(ms[:], gamma[:], s128)
                mscale[m] = ms
                # t = x*r + MAGIC (DVE: sequential fp32 rounding matches the
                # reference bit-exactly; ScalarE's multiply is lower precision)
                t = xtmp.tile([P, DIN], F32, tag="xtmp")
                nc.vector.tensor_scalar(
                    t[:], xch[:], r[:], MAGIC, op0=ALU.mult, op1=ALU.add
                )
                nc.vector.tensor_scalar(
                    t[:],
                    t[:],
                    MAGIC + (Q - 1.0),
                    MAGIC - Q,
                    op0=ALU.min,
                    op1=ALU.max,
                )
                qxn = qn_pool.tile([P, DIN], BF16, tag="qxn")
                nc.vector.tensor_scalar(qxn[:], t[:], MAGIC, None, op0=ALU.subtract)
                # transpose to [DIN, tok] layout: qt[p, k, j] = qxn[j, k*128+p]
                qt = qxt_pool.tile([P, KT, P], BF16, tag=f"qxt_{m}")
                nc.sync.dma_start_transpose(qt[:], qxn[:])
                qxt[m] = qt

            # interleave x-tile and W-panel-quad quantization so both
            # pipelines start immediately; the first chain pair needs
            # panels 0 and 1, so their quads alternate up front
            quant_x_tile(0, x0)
            quant_w_quad(0, 0)
            quant_w_quad(1, 0)
            quant_x_tile(1)
            quant_w_quad(0, 1)
            quant_w_quad(1, 1)
            quant_x_tile(2)
            quant_w_quad(0, 2)
            quant_w_quad(1, 2)
            quant_x_tile(3)
            quant_w_quad(0, 3)
            quant_w_quad(1, 3)
            quant_x_tile(4)
            quant_x_tile(5)
            quant_x_tile(6)
            quant_x_tile(7)

            # ---- matmul chains + epilogue ----
            # pair-chains: two n-blocks (2 PSUM banks) per chain; the
            # stationary qxt tile is reused across the pair, and there is a
            # single bias init / drain / store round-trip per pair.
            # Pair-0 chains are emitted BEFORE panel-2/3 quantization so the
            # chain init/drain ops outrank the bulk quant work in the
            # scheduler's priority order (drains release PSUM banks).
            def chain(n0, m, drain_on_vector):
                ps = psum_pool.tile([P, 2 * NW], F32, tag="ps")
                # pre-load bias into PSUM; the chain accumulates on top
                nc.scalar.activation(
                    ps[:],
                    biasrep[:, n0 * NW : (n0 + 2) * NW],
                    AFT.Copy,
                    bias=0.0,
                    scale=1.0,
                )
                for k in range(KT):
                    for j in range(2):
                        nc.tensor.matmul(
                            ps[:, j * NW : (j + 1) * NW],
                            qxt[m][:, k, :],
                            qwt[n0 + j][k // KQ][:, k % KQ, :],
                            start=False,
                            stop=(k == KT - 1),
                            skip_group_check=True,
                        )
                o = outp.tile([P, 2 * NW], F32, tag="o")
                nc.scalar.activation(
                    o[:], ps[:], AFT.Copy, bias=0.0, scale=mscale[m][:]
                )
                nc.sync.dma_start(
                    y_d[m * P : (m + 1) * P, n0 * NW : (n0 + 2) * NW], o[:]
                )

            for m in range(MT):
                chain(0, m, drain_on_vector=False)
            for q in range(NQ):
                quant_w_quad(2, q)
                quant_w_quad(3, q)
            for m in range(MT):
                chain(2, m, drain_on_vector=True)
    nc.compile()
    return nc


def kernel(x: np.ndarray, weight: np.ndarray, bias: np.ndarray) -> np.ndarray:
    global LAST_RESULTS
    B, S, _ = x.shape

    x2 = np.ascontiguousarray(x.reshape(B * S, DIN), dtype=np.float32)
    wt = np.ascontiguousarray(weight.T.astype(np.float32, copy=False))
    biasrep = np.ascontiguousarray(
        np.broadcast_to(bias.astype(np.float32, copy=False), (P, DOUT))
    )
    # global scalar: computed on host before sharding (see sharding note)
    scale = np.float32(max(np.mean(np.abs(weight), dtype=np.float64), EPS))
    inv_scale = np.float32(1.0 / np.float64(scale))
    s128 = np.float32(scale) / np.float32(Q)  # exact (power-of-two divide)
    consts = np.empty((P, 2), dtype=np.float32)
    consts[:, 0] = inv_scale
    consts[:, 1] = s128

    if "nc" not in _CACHE:
        _CACHE["nc"] = _build()
    nc = _CACHE["nc"]

    in_maps = [
        {
            "x": x2[i * TOK : (i + 1) * TOK],
            "wt": wt,
            "biasrep": biasrep,
            "consts": consts,
        }
        for i in range(N_CORES)
    ]
    res = run_bass_kernel_spmd(
        nc,
        in_maps,
        list(range(N_CORES)),
        trace=TRACE,
        trace_cores=list(range(N_CORES)) if TRACE else None,
    )
    LAST_RESULTS = res
    out = np.concatenate([res.results[i]["y"] for i in range(N_CORES)], axis=0)
    return np.ascontiguousarray(out.reshape(B, S, DOUT).astype(np.float32, copy=False))
